# revision 26
# baseline (speedup 1.0000x reference)
"""Multi-head attention (B=2, S=2048, E=1024, H=16) on 8 Trainium2 NeuronCores.

Sharding: heads split 2-per-core. Each core computes q/k/v projections for its
2 heads over all tokens, attention for its (2 heads x 2 batches), then a
PARTIAL output projection over ALL tokens (its 128 rows of Wo). The host sums
the 8 partial [T, E] outputs and adds bo. No inter-core collective at all, so
cores never rendezvous on device.

x is transposed on the host and shipped as xt [E, T] in bfloat16, as are all
weights: the whole matmul datapath runs bf16 operands with fp32 PSUM
accumulation (the chip is power-throttle-bound here, and bf16 MACs + halved
DMA/LDWEIGHTS cut energy). Softmax statistics stay fp32. Per-head V/ones
column split: head0 context lands on partitions 0-63, head1 on 64-127, so the
combined [128, T] context tile feeds the output projection without
cross-partition moves.
"""

import sys

if "/opt/trn_rl_repo" not in sys.path:
    sys.path.insert(0, "/opt/trn_rl_repo")

import numpy as np


def _ensure_ntff_hook():
    """bass_utils' trace path imports antenv.axon_hooks, which this image
    lacks; synthesize it (get/set pair + ctypes NTFF hook) so trace=True
    yields exec_time_ns instead of crashing."""
    import importlib
    import types

    try:
        importlib.import_module("antenv.axon_hooks")
        return
    except ImportError:
        pass
    mod = types.ModuleType("antenv.axon_hooks")
    mod._hook = None
    mod.set_axon_ntff_profile_hook = lambda h: setattr(mod, "_hook", h)
    mod.get_axon_ntff_profile_hook = lambda: mod._hook
    sys.modules["antenv.axon_hooks"] = mod
    try:
        import antenv

        antenv.axon_hooks = mod
    except ImportError:
        pass
    try:
        from trn_agent_boot.trn_boot import _ntff_profile_via_ctypes

        mod._hook = _ntff_profile_via_ctypes("/opt/axon/libaxon_pjrt.so")
    except Exception:
        pass


_ensure_ntff_hook()

B, S, E, H, DH = 2, 2048, 1024, 16, 64
T = B * S          # 4096 flattened tokens
NCORES = 8
HPC = H // NCORES  # 2 heads per core
CW = HPC * DH      # 128 projection columns per core

_CACHE = {}


def _build(debug=False):
    from contextlib import ExitStack

    import concourse.bacc as bacc
    import concourse.bass as bass
    import concourse.mybir as mybir
    import concourse.tile as tile
    from concourse.masks import make_identity

    f32 = mybir.dt.float32
    bf16 = mybir.dt.bfloat16

    nc = bacc.Bacc("TRN2", num_devices=NCORES)
    dbg = {}
    if debug:
        for name, shape in (
            ("dbg_qT", [128, T]),
            ("dbg_kT", [128, T]),
            ("dbg_vT", [128, T]),
            ("dbg_vones", [128, B * 16 * 2 * 128]),
            ("dbg_ctxn", [128, T]),
        ):
            dbg[name] = nc.declare_dram_parameter(name, shape, f32, isOutput=True)

    xt_d = nc.declare_dram_parameter("xt", [E, T], bf16, isOutput=False)
    wq_d = nc.declare_dram_parameter("wq", [E, CW], bf16, isOutput=False)
    wk_d = nc.declare_dram_parameter("wk", [E, CW], bf16, isOutput=False)
    wv_d = nc.declare_dram_parameter("wv", [E, CW], bf16, isOutput=False)
    bq_d = nc.declare_dram_parameter("bq", [CW], f32, isOutput=False)
    bk_d = nc.declare_dram_parameter("bk", [CW], f32, isOutput=False)
    bv_d = nc.declare_dram_parameter("bv", [CW], f32, isOutput=False)
    wo_d = nc.declare_dram_parameter("wo", [CW, E], bf16, isOutput=False)
    out_d = nc.declare_dram_parameter("out", [T, E], bf16, isOutput=True)

    with tile.TileContext(nc) as tc, ExitStack() as ctx:
        singles = ctx.enter_context(tc.tile_pool(name="singles", bufs=1))

        # persistent per-core activations: qT/kT [128 proj-cols, 4096 tokens]
        qT = singles.tile([128, T], bf16, tag="qT")
        kT = singles.tile([128, T], bf16, tag="kT")
        # normalized context, both heads stacked: [h0 dh | h1 dh] x tokens
        ctxn = singles.tile([128, T], bf16, tag="ctxn")

        ident = singles.tile([128, 128], f32, tag="ident")
        make_identity(nc, ident)
        ident_b = singles.tile([128, 128], bf16, tag="identb")
        nc.vector.tensor_copy(out=ident_b, in_=ident)

        # --- weights / biases (DMA straight into bf16 tiles) ---
        wq_sb = singles.tile([128, 8, CW], bf16, tag="wq")
        wk_sb = singles.tile([128, 8, CW], bf16, tag="wk")
        wv_sb = singles.tile([128, 8, CW], bf16, tag="wv")
        wo_sb = singles.tile([128, E], bf16, tag="wo")
        for w_sb, w_d in ((wq_sb, wq_d), (wk_sb, wk_d), (wv_sb, wv_d)):
            nc.sync.dma_start(
                out=w_sb, in_=w_d.ap().rearrange("(o p) c -> p o c", p=128)
            )
        bq_sb = singles.tile([128, 1], f32, tag="bq")
        bk_sb = singles.tile([128, 1], f32, tag="bk")
        bv_sb = singles.tile([128, 1], f32, tag="bv")
        for b_sb, b_d in ((bq_sb, bq_d), (bk_sb, bk_d), (bv_sb, bv_d)):
            nc.sync.dma_start(out=b_sb, in_=b_d.ap().rearrange("(p o) -> p o", o=1))

        # constant 1/64 stationary operand for the PE row-sum broadcast
        const64 = singles.tile([128, 128], bf16, tag="c64")
        nc.vector.tensor_scalar(
            out=const64,
            in0=ident,
            scalar1=0.0,
            scalar2=1.0 / 64.0,
            op0=mybir.AluOpType.mult,
            op1=mybir.AluOpType.add,
        )

        # [token_p, b, jtile, head, 128]: h0 = [64 v | 64 ones],
        # h1 = [64 ones | 64 v] so ctx lands on the head's own partition half.
        v_ones = singles.tile([128, B, 16, 2, 128], bf16, tag="vones")

        vT_pool = ctx.enter_context(tc.tile_pool(name="vT", bufs=1))
        vT = vT_pool.tile([128, T], bf16, tag="vT")

        # --- phase 1: project q/k/v from pre-transposed x (per 512-tok chunk)
        with (
            tc.tile_pool(name="ph1x", bufs=2) as ph1x,
            tc.tile_pool(name="projps", bufs=3, space="PSUM") as projps,
        ):
            for tchunk in range(8):
                tsl = slice(tchunk * 512, (tchunk + 1) * 512)
                xT_sb = ph1x.tile([128, 8, 512], bf16, tag="xT")
                nc.sync.dma_start(
                    out=xT_sb,
                    in_=xt_d.ap()[:, tsl].rearrange("(o p) t -> p o t", p=128),
                )
                for w_sb, b_sb, dstT in (
                    (wq_sb, bq_sb, qT),
                    (wk_sb, bk_sb, kT),
                    (wv_sb, bv_sb, vT),
                ):
                    ps_p = projps.tile([128, 512], f32, tag="proj")
                    for dc in range(8):
                        nc.tensor.matmul(
                            ps_p,
                            w_sb[:, dc, :],
                            xT_sb[:, dc, :],
                            start=(dc == 0),
                            stop=(dc == 7),
                        )
                    nc.vector.tensor_scalar_add(
                        out=dstT[:, tsl], in0=ps_p, scalar1=b_sb
                    )

        # wo is only needed by the output projection; load it after the
        # xt chunks so it doesn't delay the first projection matmuls.
        nc.sync.dma_start(out=wo_sb, in_=wo_d.ap())

        # --- phase 2: transpose vT into natural layout (per-head halves) ---
        with tc.tile_pool(name="vtps", bufs=2, space="PSUM") as vtps:
            for b in range(B):
                for j in range(16):
                    jsl = slice(b * S + j * 128, b * S + (j + 1) * 128)
                    ps_v = vtps.tile([128, 128], f32, tag="vt")
                    nc.tensor.matmul(
                        ps_v, vT[:, jsl], ident_b, start=True, stop=True
                    )
                    nc.vector.tensor_copy(
                        out=v_ones[:, b, j, 0, 0:64], in_=ps_v[:, 0:64]
                    )
                    nc.vector.tensor_copy(
                        out=v_ones[:, b, j, 1, 64:128], in_=ps_v[:, 64:128]
                    )

        # ones planes, written once (tensor_scalar writes exact 1.0 in bf16;
        # in0 values are irrelevant, qT is just a ready same-shape source)
        for h, csl in ((0, slice(64, 128)), (1, slice(0, 64))):
            nc.vector.tensor_scalar(
                out=v_ones[:, :, :, h, csl],
                in0=qT[:, 0:2048].rearrange("p (a b c) -> p a b c", a=2, b=16),
                scalar1=0.0,
                scalar2=1.0,
                op0=mybir.AluOpType.mult,
                op1=mybir.AluOpType.add,
            )

        # --- phase 3: attention (scores^T -> exp -> ctx^T + row-sums) ---
        with (
            tc.tile_pool(name="att", bufs=4) as att,
            tc.tile_pool(name="dv", bufs=4) as dv,
            tc.tile_pool(name="stps", bufs=2, space="PSUM") as stps,
            tc.tile_pool(name="ctxps", bufs=1, space="PSUM") as ctxps,
            tc.tile_pool(name="lrps", bufs=2, space="PSUM") as lrps,
        ):
            for b in range(B):
                for half in range(2):
                    i0 = b * S + half * 1024
                    for h in range(2):
                        hr = slice(64 * h, 64 * h + 64)
                        # partition halves: where this head's dh and sums live
                        c0, c1 = (0, 64) if h == 0 else (64, 128)   # ctx rows
                        s0, s1 = (64, 128) if h == 0 else (0, 64)   # sum rows
                        ctx_ps = ctxps.tile([128, 2, 512], f32, tag="ctx")
                        # software-pipelined j loop: score MMs for j+1 are
                        # emitted before ctx MMs for j, so the PE streams
                        # scores while the ACT engine exponentiates.
                        pending = None
                        for j in range(16):
                            jsl = slice(b * S + j * 128, b * S + (j + 1) * 128)
                            st = stps.tile([128, 2, 512], f32, tag="st")
                            for s in range(2):
                                nc.tensor.matmul(
                                    st[:, s, :],
                                    kT[hr, jsl],
                                    qT[hr, i0 + 512 * s : i0 + 512 * (s + 1)],
                                    start=True,
                                    stop=True,
                                )
                            expst = att.tile([128, 1024], bf16, tag="expst")
                            nc.scalar.activation(
                                out=expst,
                                in_=st.rearrange("p a b -> p (a b)"),
                                func=mybir.ActivationFunctionType.Exp,
                                scale=0.125,
                            )
                            if pending is not None:
                                pexp, pj = pending
                                for s in range(2):
                                    nc.tensor.matmul(
                                        ctx_ps[:, s, :],
                                        v_ones[:, b, pj, h, :],
                                        pexp[:, 512 * s : 512 * (s + 1)],
                                        start=(pj == 0),
                                        stop=False,
                                    )
                            pending = (expst, j)
                        pexp, pj = pending
                        for s in range(2):
                            nc.tensor.matmul(
                                ctx_ps[:, s, :],
                                v_ones[:, b, pj, h, :],
                                pexp[:, 512 * s : 512 * (s + 1)],
                                start=False,
                                stop=True,
                            )
                        for s in range(2):
                            isl = slice(i0 + 512 * s, i0 + 512 * (s + 1))
                            # row-sums live on the opposite partition half;
                            # PE-broadcast them onto this head's half (sum of
                            # 64 identical rows x 1/64), then recip + multiply.
                            l_sb = dv.tile([128, 512], bf16, tag="lsb")
                            nc.vector.tensor_copy(
                                out=l_sb[s0:s1, :], in_=ctx_ps[s0:s1, s, :]
                            )
                            lr_ps = lrps.tile([128, 512], f32, tag="lrbc")
                            nc.tensor.matmul(
                                lr_ps,
                                const64[s0:s1, :],
                                l_sb[s0:s1, :],
                                start=True,
                                stop=True,
                            )
                            # full-tile recip: the custom DVE uop mis-executes
                            # on a base-partition-64 slice, so compute all 128
                            # partitions (lr_ps is fully written) and slice.
                            lr = dv.tile([128, 512], f32, tag="lr")
                            nc.vector.reciprocal_approx_fast(out=lr, in_=lr_ps)
                            nc.vector.tensor_mul(
                                out=ctxn[c0:c1, isl],
                                in0=ctx_ps[c0:c1, s, :],
                                in1=lr[c0:c1, :],
                            )

        if debug:
            nc.gpsimd.dma_start(out=dbg["dbg_qT"].ap(), in_=qT)
            nc.gpsimd.dma_start(out=dbg["dbg_kT"].ap(), in_=kT)
            nc.gpsimd.dma_start(out=dbg["dbg_vT"].ap(), in_=vT)
            nc.gpsimd.dma_start(
                out=dbg["dbg_vones"].ap(),
                in_=v_ones.rearrange("p b j h c -> p (b j h c)"),
            )
            nc.gpsimd.dma_start(out=dbg["dbg_ctxn"].ap(), in_=ctxn)

        # --- phase 4: partial output projection over ALL tokens ---
        with (
            tc.tile_pool(name="ph4", bufs=3) as ph4,
            tc.tile_pool(name="ph4ps", bufs=2, space="PSUM") as ph4ps,
        ):
            for tt in range(32):
                ps_o = ph4ps.tile([128, 2, 512], f32, tag="o")
                for ec in range(2):
                    nc.tensor.matmul(
                        ps_o[:, ec, :],
                        ctxn[:, tt * 128 : (tt + 1) * 128],
                        wo_sb[:, ec * 512 : (ec + 1) * 512],
                        start=True,
                        stop=True,
                    )
                o_sb = ph4.tile([128, 2, 512], bf16, tag="osb")
                # alternate PSUM->SBUF copies between DVE and ACT (both
                # engines are otherwise idle in this phase)
                if tt % 2 == 0:
                    nc.vector.tensor_copy(out=o_sb, in_=ps_o)
                else:
                    nc.scalar.activation(
                        out=o_sb.rearrange("p a b -> p (a b)"),
                        in_=ps_o.rearrange("p a b -> p (a b)"),
                        func=mybir.ActivationFunctionType.Copy,
                    )
                nc.sync.dma_start(
                    out=out_d.ap()[tt * 128 : (tt + 1) * 128, :],
                    in_=o_sb.rearrange("p a b -> p (a b)"),
                )

    nc.finalize()
    return nc


def _get_nc():
    import os

    debug = bool(int(os.environ.get("MHA_DEBUG", "0")))
    key = ("nc", debug)
    if key not in _CACHE:
        _CACHE[key] = _build(debug)
    return _CACHE[key]


def kernel(x, Wq, bq, Wk, bk, Wv, bv, Wo, bo, **_ignored):
    import ml_dtypes

    from concourse.bass_utils import run_bass_kernel_spmd

    bf = ml_dtypes.bfloat16
    x = np.asarray(x, dtype=np.float32).reshape(T, E)
    xt = np.ascontiguousarray(x.T).astype(bf)  # [E, T] bf16
    Wq = np.asarray(Wq, dtype=np.float32).astype(bf)
    Wk = np.asarray(Wk, dtype=np.float32).astype(bf)
    Wv = np.asarray(Wv, dtype=np.float32).astype(bf)
    Wo = np.asarray(Wo, dtype=np.float32).astype(bf)
    bq = np.asarray(bq, dtype=np.float32)
    bk = np.asarray(bk, dtype=np.float32)
    bv = np.asarray(bv, dtype=np.float32)
    bo = np.ascontiguousarray(np.asarray(bo, dtype=np.float32))

    in_maps = []
    for c in range(NCORES):
        csl = slice(c * CW, (c + 1) * CW)
        in_maps.append(
            {
                "xt": xt,
                "wq": np.ascontiguousarray(Wq[:, csl]),
                "wk": np.ascontiguousarray(Wk[:, csl]),
                "wv": np.ascontiguousarray(Wv[:, csl]),
                "bq": np.ascontiguousarray(bq[csl]),
                "bk": np.ascontiguousarray(bk[csl]),
                "bv": np.ascontiguousarray(bv[csl]),
                "wo": np.ascontiguousarray(Wo[csl, :]),
            }
        )

    nc = _get_nc()
    import os

    trace = bool(int(os.environ.get("MHA_TRACE", "0")))
    res = run_bass_kernel_spmd(
        nc, in_maps, core_ids=list(range(NCORES)), trace=trace
    )
    if trace:
        _CACHE["last_results"] = res
    out = res.results[0]["out"].astype(np.float32)
    for c in range(1, NCORES):
        out += res.results[c]["out"].astype(np.float32)
    out += bo
    return out.reshape(B, S, E)


# revision 27
# speedup vs baseline: 1.1734x; 1.1734x over previous
"""Multi-head attention (B=2, S=2048, E=1024, H=16) on 8 Trainium2 NeuronCores.

Sharding: heads split 2-per-core. Each core computes q/k/v projections for its
2 heads over all tokens, attention for its (2 heads x 2 batches), then a
PARTIAL output projection over ALL tokens (its 128 rows of Wo). The host sums
the 8 partial [T, E] outputs and adds bo. No inter-core collective at all, so
cores never rendezvous on device.

x is transposed on the host and shipped as xt [E, T] in bfloat16, as are all
weights: the whole matmul datapath runs bf16 operands with fp32 PSUM
accumulation (the chip is power-throttle-bound here, and bf16 MACs + halved
DMA/LDWEIGHTS cut energy). Softmax statistics stay fp32. Per-head V/ones
column split: head0 context lands on partitions 0-63, head1 on 64-127, so the
combined [128, T] context tile feeds the output projection without
cross-partition moves.
"""

import sys

if "/opt/trn_rl_repo" not in sys.path:
    sys.path.insert(0, "/opt/trn_rl_repo")

import numpy as np


def _ensure_ntff_hook():
    """bass_utils' trace path imports antenv.axon_hooks, which this image
    lacks; synthesize it (get/set pair + ctypes NTFF hook) so trace=True
    yields exec_time_ns instead of crashing."""
    import importlib
    import types

    try:
        importlib.import_module("antenv.axon_hooks")
        return
    except ImportError:
        pass
    mod = types.ModuleType("antenv.axon_hooks")
    mod._hook = None
    mod.set_axon_ntff_profile_hook = lambda h: setattr(mod, "_hook", h)
    mod.get_axon_ntff_profile_hook = lambda: mod._hook
    sys.modules["antenv.axon_hooks"] = mod
    try:
        import antenv

        antenv.axon_hooks = mod
    except ImportError:
        pass
    try:
        from trn_agent_boot.trn_boot import _ntff_profile_via_ctypes

        mod._hook = _ntff_profile_via_ctypes("/opt/axon/libaxon_pjrt.so")
    except Exception:
        pass


_ensure_ntff_hook()

B, S, E, H, DH = 2, 2048, 1024, 16, 64
T = B * S          # 4096 flattened tokens
NCORES = 8
HPC = H // NCORES  # 2 heads per core
CW = HPC * DH      # 128 projection columns per core

_CACHE = {}


def _build(debug=False):
    from contextlib import ExitStack

    import concourse.bacc as bacc
    import concourse.bass as bass
    import concourse.mybir as mybir
    import concourse.tile as tile
    from concourse.masks import make_identity

    f32 = mybir.dt.float32
    bf16 = mybir.dt.bfloat16

    nc = bacc.Bacc("TRN2", num_devices=NCORES)
    dbg = {}
    if debug:
        for name, shape in (
            ("dbg_qT", [128, T]),
            ("dbg_kT", [128, T]),
            ("dbg_vT", [128, T]),
            ("dbg_vones", [128, B * 16 * 2 * 128]),
            ("dbg_ctxn", [128, T]),
        ):
            dbg[name] = nc.declare_dram_parameter(name, shape, f32, isOutput=True)

    xt_d = nc.declare_dram_parameter("xt", [E, T], bf16, isOutput=False)
    wq_d = nc.declare_dram_parameter("wq", [E, CW], bf16, isOutput=False)
    wk_d = nc.declare_dram_parameter("wk", [E, CW], bf16, isOutput=False)
    wv_d = nc.declare_dram_parameter("wv", [E, CW], bf16, isOutput=False)
    bq_d = nc.declare_dram_parameter("bq", [CW], f32, isOutput=False)
    bk_d = nc.declare_dram_parameter("bk", [CW], f32, isOutput=False)
    bv_d = nc.declare_dram_parameter("bv", [CW], f32, isOutput=False)
    wo_d = nc.declare_dram_parameter("wo", [CW, E], bf16, isOutput=False)
    out_d = nc.declare_dram_parameter("out", [T, E], bf16, isOutput=True)

    with tile.TileContext(nc) as tc, ExitStack() as ctx:
        singles = ctx.enter_context(tc.tile_pool(name="singles", bufs=1))

        # persistent per-core activations: qT/kT [128 proj-cols, 4096 tokens]
        qT = singles.tile([128, T], bf16, tag="qT")
        kT = singles.tile([128, T], bf16, tag="kT")
        # normalized context, both heads stacked: [h0 dh | h1 dh] x tokens
        ctxn = singles.tile([128, T], bf16, tag="ctxn")

        ident = singles.tile([128, 128], f32, tag="ident")
        make_identity(nc, ident)
        ident_b = singles.tile([128, 128], bf16, tag="identb")
        nc.vector.tensor_copy(out=ident_b, in_=ident)

        # --- weights / biases (DMA straight into bf16 tiles) ---
        wq_sb = singles.tile([128, 8, CW], bf16, tag="wq")
        wk_sb = singles.tile([128, 8, CW], bf16, tag="wk")
        wv_sb = singles.tile([128, 8, CW], bf16, tag="wv")
        wo_sb = singles.tile([128, E], bf16, tag="wo")
        for w_sb, w_d in ((wq_sb, wq_d), (wk_sb, wk_d), (wv_sb, wv_d)):
            nc.sync.dma_start(
                out=w_sb, in_=w_d.ap().rearrange("(o p) c -> p o c", p=128)
            )
        bq_sb = singles.tile([128, 1], f32, tag="bq")
        bk_sb = singles.tile([128, 1], f32, tag="bk")
        bv_sb = singles.tile([128, 1], f32, tag="bv")
        for b_sb, b_d in ((bq_sb, bq_d), (bk_sb, bk_d), (bv_sb, bv_d)):
            nc.sync.dma_start(out=b_sb, in_=b_d.ap().rearrange("(p o) -> p o", o=1))

        # constant 1/64 stationary operand for the PE row-sum broadcast
        const64 = singles.tile([128, 128], bf16, tag="c64")
        nc.vector.tensor_scalar(
            out=const64,
            in0=ident,
            scalar1=0.0,
            scalar2=1.0 / 64.0,
            op0=mybir.AluOpType.mult,
            op1=mybir.AluOpType.add,
        )

        # [token_p, b, jtile, head, 128]: h0 = [64 v | 64 ones],
        # h1 = [64 ones | 64 v] so ctx lands on the head's own partition half.
        v_ones = singles.tile([128, B, 16, 2, 128], bf16, tag="vones")

        vT_pool = ctx.enter_context(tc.tile_pool(name="vT", bufs=1))
        vT = vT_pool.tile([128, T], bf16, tag="vT")

        # --- phase 1: project q/k/v from pre-transposed x (per 512-tok chunk)
        with (
            tc.tile_pool(name="ph1x", bufs=2) as ph1x,
            tc.tile_pool(name="projps", bufs=3, space="PSUM") as projps,
        ):
            for tchunk in range(8):
                tsl = slice(tchunk * 512, (tchunk + 1) * 512)
                xT_sb = ph1x.tile([128, 8, 512], bf16, tag="xT")
                nc.sync.dma_start(
                    out=xT_sb,
                    in_=xt_d.ap()[:, tsl].rearrange("(o p) t -> p o t", p=128),
                )
                for w_sb, b_sb, dstT in (
                    (wq_sb, bq_sb, qT),
                    (wk_sb, bk_sb, kT),
                    (wv_sb, bv_sb, vT),
                ):
                    ps_p = projps.tile([128, 512], f32, tag="proj")
                    for dc in range(8):
                        nc.tensor.matmul(
                            ps_p,
                            w_sb[:, dc, :],
                            xT_sb[:, dc, :],
                            start=(dc == 0),
                            stop=(dc == 7),
                        )
                    nc.vector.tensor_scalar_add(
                        out=dstT[:, tsl], in0=ps_p, scalar1=b_sb
                    )

        # wo is only needed by the output projection; load it after the
        # xt chunks so it doesn't delay the first projection matmuls.
        nc.sync.dma_start(out=wo_sb, in_=wo_d.ap())

        # --- phase 2: transpose vT into natural layout (per-head halves) ---
        with tc.tile_pool(name="vtps", bufs=2, space="PSUM") as vtps:
            for b in range(B):
                for j in range(16):
                    jsl = slice(b * S + j * 128, b * S + (j + 1) * 128)
                    ps_v = vtps.tile([128, 128], f32, tag="vt")
                    nc.tensor.matmul(
                        ps_v, vT[:, jsl], ident_b, start=True, stop=True
                    )
                    nc.vector.tensor_copy(
                        out=v_ones[:, b, j, 0, 0:64], in_=ps_v[:, 0:64]
                    )
                    nc.vector.tensor_copy(
                        out=v_ones[:, b, j, 1, 64:128], in_=ps_v[:, 64:128]
                    )

        # ones planes, written once (tensor_scalar writes exact 1.0 in bf16;
        # in0 values are irrelevant, qT is just a ready same-shape source)
        for h, csl in ((0, slice(64, 128)), (1, slice(0, 64))):
            nc.vector.tensor_scalar(
                out=v_ones[:, :, :, h, csl],
                in0=qT[:, 0:2048].rearrange("p (a b c) -> p a b c", a=2, b=16),
                scalar1=0.0,
                scalar2=1.0,
                op0=mybir.AluOpType.mult,
                op1=mybir.AluOpType.add,
            )

        # --- phase 3: attention (scores^T -> exp -> ctx^T + row-sums) ---
        with (
            tc.tile_pool(name="att", bufs=4) as att,
            tc.tile_pool(name="dv", bufs=4) as dv,
            tc.tile_pool(name="stps", bufs=2, space="PSUM") as stps,
            tc.tile_pool(name="ctxps", bufs=1, space="PSUM") as ctxps,
            tc.tile_pool(name="lrps", bufs=2, space="PSUM") as lrps,
        ):
            for b in range(B):
                for half in range(2):
                    i0 = b * S + half * 1024
                    for h in range(2):
                        hr = slice(64 * h, 64 * h + 64)
                        # partition halves: where this head's dh and sums live
                        c0, c1 = (0, 64) if h == 0 else (64, 128)   # ctx rows
                        s0, s1 = (64, 128) if h == 0 else (0, 64)   # sum rows
                        ctx_ps = ctxps.tile([128, 2, 512], f32, tag="ctx")
                        # software-pipelined j loop: score MMs for j+1 are
                        # emitted before ctx MMs for j, so the PE streams
                        # scores while the ACT engine exponentiates.
                        pending = None
                        for j in range(16):
                            jsl = slice(b * S + j * 128, b * S + (j + 1) * 128)
                            st = stps.tile([128, 2, 512], f32, tag="st")
                            for s in range(2):
                                nc.tensor.matmul(
                                    st[:, s, :],
                                    kT[hr, jsl],
                                    qT[hr, i0 + 512 * s : i0 + 512 * (s + 1)],
                                    start=True,
                                    stop=True,
                                )
                            expst = att.tile([128, 1024], bf16, tag="expst")
                            nc.scalar.activation(
                                out=expst,
                                in_=st.rearrange("p a b -> p (a b)"),
                                func=mybir.ActivationFunctionType.Exp,
                                scale=0.125,
                            )
                            if pending is not None:
                                pexp, pj = pending
                                for s in range(2):
                                    nc.tensor.matmul(
                                        ctx_ps[:, s, :],
                                        v_ones[:, b, pj, h, :],
                                        pexp[:, 512 * s : 512 * (s + 1)],
                                        start=(pj == 0),
                                        stop=False,
                                    )
                            pending = (expst, j)
                        pexp, pj = pending
                        for s in range(2):
                            nc.tensor.matmul(
                                ctx_ps[:, s, :],
                                v_ones[:, b, pj, h, :],
                                pexp[:, 512 * s : 512 * (s + 1)],
                                start=False,
                                stop=True,
                            )
                        for s in range(2):
                            isl = slice(i0 + 512 * s, i0 + 512 * (s + 1))
                            # row-sums live on the opposite partition half;
                            # PE-broadcast them onto this head's half (sum of
                            # 64 identical rows x 1/64), then recip + multiply.
                            l_sb = dv.tile([128, 512], bf16, tag="lsb")
                            nc.vector.tensor_copy(
                                out=l_sb[s0:s1, :], in_=ctx_ps[s0:s1, s, :]
                            )
                            lr_ps = lrps.tile([128, 512], f32, tag="lrbc")
                            nc.tensor.matmul(
                                lr_ps,
                                const64[s0:s1, :],
                                l_sb[s0:s1, :],
                                start=True,
                                stop=True,
                            )
                            # full-tile recip: the custom DVE uop mis-executes
                            # on a base-partition-64 slice, so compute all 128
                            # partitions (lr_ps is fully written) and slice.
                            lr = dv.tile([128, 512], f32, tag="lr")
                            nc.vector.reciprocal_approx_fast(out=lr, in_=lr_ps)
                            nc.vector.tensor_mul(
                                out=ctxn[c0:c1, isl],
                                in0=ctx_ps[c0:c1, s, :],
                                in1=lr[c0:c1, :],
                            )

        if debug:
            nc.gpsimd.dma_start(out=dbg["dbg_qT"].ap(), in_=qT)
            nc.gpsimd.dma_start(out=dbg["dbg_kT"].ap(), in_=kT)
            nc.gpsimd.dma_start(out=dbg["dbg_vT"].ap(), in_=vT)
            nc.gpsimd.dma_start(
                out=dbg["dbg_vones"].ap(),
                in_=v_ones.rearrange("p b j h c -> p (b j h c)"),
            )
            nc.gpsimd.dma_start(out=dbg["dbg_ctxn"].ap(), in_=ctxn)

        # --- phase 4: partial output projection over ALL tokens ---
        with (
            tc.tile_pool(name="ph4", bufs=3) as ph4,
            tc.tile_pool(name="ph4ps", bufs=2, space="PSUM") as ph4ps,
        ):
            for tt in range(32):
                ps_o = ph4ps.tile([128, 2, 512], f32, tag="o")
                for ec in range(2):
                    nc.tensor.matmul(
                        ps_o[:, ec, :],
                        ctxn[:, tt * 128 : (tt + 1) * 128],
                        wo_sb[:, ec * 512 : (ec + 1) * 512],
                        start=True,
                        stop=True,
                    )
                o_sb = ph4.tile([128, 2, 512], bf16, tag="osb")
                nc.vector.tensor_copy(out=o_sb, in_=ps_o)
                nc.sync.dma_start(
                    out=out_d.ap()[tt * 128 : (tt + 1) * 128, :],
                    in_=o_sb.rearrange("p a b -> p (a b)"),
                )

    nc.finalize()
    return nc


def _get_nc():
    import os

    debug = bool(int(os.environ.get("MHA_DEBUG", "0")))
    key = ("nc", debug)
    if key not in _CACHE:
        _CACHE[key] = _build(debug)
    return _CACHE[key]


def kernel(x, Wq, bq, Wk, bk, Wv, bv, Wo, bo, **_ignored):
    import ml_dtypes

    from concourse.bass_utils import run_bass_kernel_spmd

    bf = ml_dtypes.bfloat16
    x = np.asarray(x, dtype=np.float32).reshape(T, E)
    xt = np.ascontiguousarray(x.T).astype(bf)  # [E, T] bf16
    Wq = np.asarray(Wq, dtype=np.float32).astype(bf)
    Wk = np.asarray(Wk, dtype=np.float32).astype(bf)
    Wv = np.asarray(Wv, dtype=np.float32).astype(bf)
    Wo = np.asarray(Wo, dtype=np.float32).astype(bf)
    bq = np.asarray(bq, dtype=np.float32)
    bk = np.asarray(bk, dtype=np.float32)
    bv = np.asarray(bv, dtype=np.float32)
    bo = np.ascontiguousarray(np.asarray(bo, dtype=np.float32))

    in_maps = []
    for c in range(NCORES):
        csl = slice(c * CW, (c + 1) * CW)
        in_maps.append(
            {
                "xt": xt,
                "wq": np.ascontiguousarray(Wq[:, csl]),
                "wk": np.ascontiguousarray(Wk[:, csl]),
                "wv": np.ascontiguousarray(Wv[:, csl]),
                "bq": np.ascontiguousarray(bq[csl]),
                "bk": np.ascontiguousarray(bk[csl]),
                "bv": np.ascontiguousarray(bv[csl]),
                "wo": np.ascontiguousarray(Wo[csl, :]),
            }
        )

    nc = _get_nc()
    import os

    trace = bool(int(os.environ.get("MHA_TRACE", "0")))
    res = run_bass_kernel_spmd(
        nc, in_maps, core_ids=list(range(NCORES)), trace=trace
    )
    if trace:
        _CACHE["last_results"] = res
    out = res.results[0]["out"].astype(np.float32)
    for c in range(1, NCORES):
        out += res.results[c]["out"].astype(np.float32)
    out += bo
    return out.reshape(B, S, E)
